# revision 11
# baseline (speedup 1.0000x reference)
"""BiLSTM-CRF loss kernel for Trainium2, data-parallel over batch on 8 cores.

Layout (per core, batch slice Bc=16):
- tokens are t-major: tok = t*Bc + b
- LSTM state transposed: hT [128 partitions = hidden-within-half, 32 cols =
  (half, b)]; gates as gatesT [128, 128] in PSUM via matmul(lhsT=W_hh^T tile,
  rhs=hT half-slice), M-tile (gate-block) order (i0,i1,f0,f1,o0,o1,g0,g1) so
  sigmoid covers cols 0:96, tanh 96:128 after the xp add.
- backward direction walks t descending over the same buffers, forcing c to
  zero while t >= seq_len (h then follows; stored outputs at padded t are 0).
- CRF: alphaT [20, 16]; alpha_new = emit + ref + log(expT.T @ exp(alpha-ref)),
  ref = broadcast of alpha row 0 via a K=1 matmul; masked via copy_predicated.
- gold score entirely on device via one-hot matmuls (emissions + transition
  pairs + final END transition).
"""

import numpy as np
import ml_dtypes

V, D, HH, HD, T, B, NT = 30000, 256, 256, 512, 512, 128, 20
BEGIN, END, NEG = 18, 19, -99999.0
NCORES = 8
BC = B // NCORES          # 16 sequences per core
SEG = 32                  # scan segment (steps)
CHAINS = ("l0f", "l0b", "l1f", "l1b")
KT_IN = {"l0f": 2, "l0b": 2, "l1f": 4, "l1b": 4}

_BF16 = ml_dtypes.bfloat16

# gate-row permutation: pytorch (i,f,g,o) blocks of HH -> (i,f,o,g)
_GATE_PERM = np.concatenate([
    np.arange(0, 256),        # i
    np.arange(256, 512),      # f
    np.arange(768, 1024),     # o
    np.arange(512, 768),      # g
])


def _pack_lhsT(w):
    """w: [1024, Din] (gate-permuted rows) -> [128, KT*8*128] f32 so that
    packed[:, kt*8*128 + m*128 + col] on partition p = w[m*128+col, kt*128+p]."""
    din = w.shape[1]
    kt = din // 128
    a = w.reshape(8, 128, kt, 128)          # [m, col, kt, part]
    a = a.transpose(3, 2, 0, 1)             # [part, kt, m, col]
    return np.ascontiguousarray(a.reshape(128, kt * 8 * 128))


def _patched_tc():
    """TileContext with final-drain sem waits split into single-wait nops
    (walrus here caps sync-wait commands per CTRL instruction at 1)."""
    import concourse.mybir as mybir
    import concourse.tile as tile
    from concourse.tile import ScopedClock

    class TC(tile.TileContext):
        def _drain_and_barrier(self, tick_clock, wait_clock):
            nc = self.nc
            drain_inst = nc.sync.drain()
            wait_clock.add_sem_waits(
                drain_inst.ins, ScopedClock({None: tick_clock.global_clock}))
            si = drain_inst.ins.sync_info
            if si is not None and si.on_wait and len(si.on_wait) > 1:
                waits = list(si.on_wait)
                drain_inst.ins.sync_info = mybir.SyncInfo(
                    on_wait=[], on_update=list(si.on_update))
                bb = nc.cur_bb.bb
                bb.instructions.pop()
                for w in waits:
                    n = nc.sync.nop(nofuse=True)
                    n.ins.sync_info = mybir.SyncInfo(on_wait=[w], on_update=[])
                bb.add_instruction(drain_inst.ins)
            nc.all_engine_barrier()
            assert self.sems is not None
            popped = nc._tile_sem_poison_stack.pop()
            assert popped is self._sem_poison
            nc.clear_and_free_semaphores(list(self.sems.allocated().values()))
            nc.all_engine_barrier()

    return TC


def _split_waits(nc, limit=1):
    """walrus on this stack rejects >limit sem-wait commands per instruction;
    hoist excess waits into preceding same-engine NoOps."""
    import concourse.mybir as mybir
    for fn in nc.m.functions:
        for bb in fn.blocks:
            out = []
            changed = False
            for inst in bb.instructions:
                si = inst.sync_info
                if si is not None and si.on_wait and len(si.on_wait) > limit:
                    waits = list(si.on_wait)
                    for j, w in enumerate(waits[:-limit]):
                        ni = mybir.InstNoOp(
                            name=f"{inst.name}-wsplit{j}", ins=[], outs=[])
                        ni.engine = inst.engine
                        ni.sync_info = mybir.SyncInfo(on_wait=[w], on_update=[])
                        out.append(ni)
                    inst.sync_info = mybir.SyncInfo(
                        on_wait=waits[-limit:], on_update=list(si.on_update))
                    changed = True
                out.append(inst)
            if changed:
                bb.instructions[:] = out


def build_program(t_steps=T, stage=4, dbg=False):
    import contextlib
    import concourse.bass as bass
    import concourse.mybir as mybir
    from concourse.masks import make_identity

    dt = mybir.dt
    AF = mybir.ActivationFunctionType
    ALU = mybir.AluOpType

    nseg = t_steps // SEG
    ntok = t_steps * BC

    nc = bass.Bass("TRN2", target_bir_lowering=False, debug=False)

    # ---------------- inputs ----------------
    sent_tm = nc.dram_tensor("sent_tm", [ntok, 1], dt.int32, kind="ExternalInput")
    embed = nc.dram_tensor("embed", [V, D], dt.float32, kind="ExternalInput")
    wih = {c: nc.dram_tensor(f"wih_{c}", [128, KT_IN[c] * 8 * 128], dt.bfloat16,
                             kind="ExternalInput") for c in CHAINS}
    whh = {c: nc.dram_tensor(f"whh_{c}", [128, 2 * 8 * 128], dt.bfloat16,
                             kind="ExternalInput") for c in CHAINS}
    bias = {c: nc.dram_tensor(f"bias_{c}", [128, 8], dt.float32,
                              kind="ExternalInput") for c in CHAINS}
    mask_scan = nc.dram_tensor("mask_scan", [128, t_steps * 32], dt.bfloat16,
                               kind="ExternalInput")
    transT = nc.dram_tensor("transT", [NT, NT], dt.float32, kind="ExternalInput")
    trans_colB = nc.dram_tensor("trans_colB", [NT, 1], dt.float32, kind="ExternalInput")
    wgT = nc.dram_tensor("wgT", [128, 4 * NT], dt.bfloat16, kind="ExternalInput")
    out_wT = nc.dram_tensor("out_wT", [128, 4 * NT], dt.bfloat16, kind="ExternalInput")
    ln_b_k = nc.dram_tensor("ln_b_k", [128, 4], dt.bfloat16, kind="ExternalInput")
    out_b_col = nc.dram_tensor("out_b_col", [NT, 1], dt.float32, kind="ExternalInput")
    oht = nc.dram_tensor("oht", [NT, ntok], dt.bfloat16, kind="ExternalInput")
    oh2 = nc.dram_tensor("oh2", [512, ntok], dt.bfloat16, kind="ExternalInput")
    trans_flat = nc.dram_tensor("trans_flat", [128, 4], dt.bfloat16, kind="ExternalInput")
    crf_mask = nc.dram_tensor("crf_mask", [NT, ntok], dt.uint8, kind="ExternalInput")
    last_oh = nc.dram_tensor("last_oh", [NT, BC], dt.bfloat16, kind="ExternalInput")

    # ---------------- outputs ----------------
    fscore_out = nc.dram_tensor("fscore_out", [1, BC], dt.float32, kind="ExternalOutput")
    gold_out = nc.dram_tensor("gold_out", [1, BC], dt.float32, kind="ExternalOutput")

    ikind = "ExternalOutput" if dbg else "Internal"
    x0T = [nc.dram_tensor(f"x0T_k{k}", [128, ntok], dt.bfloat16, kind=ikind)
           for k in range(2)]
    hbuf = {c: [nc.dram_tensor(f"h_{c}_k{k}", [128, ntok], dt.bfloat16, kind=ikind)
                for k in range(2)] for c in CHAINS}
    dbg_feats = None
    if dbg and stage >= 3:
        dbg_feats = nc.dram_tensor("dbg_feats", [NT, ntok], dt.float32,
                                   kind="ExternalOutput")

    TCls = _patched_tc()
    with TCls(nc) as tc, contextlib.ExitStack() as ctx:
        # long-lived SBUF pools
        consts = ctx.enter_context(tc.tile_pool(name="consts", bufs=1))
        wpool = ctx.enter_context(tc.tile_pool(name="wpool", bufs=1))
        fpool = ctx.enter_context(tc.tile_pool(name="fpool", bufs=1))

        # ---------- constants ----------
        ident = consts.tile([128, 128], dt.float32)
        make_identity(nc, ident[:])
        ones_row = consts.tile([128, 32], dt.float32)   # [0:1, 0:20] = lhsT K=1
        nc.vector.memset(ones_row[:], 1.0)
        ones_col_bf = consts.tile([128, 1], dt.bfloat16)
        nc.vector.memset(ones_col_bf[:], 1.0)
        ones20_col = consts.tile([128, 1], dt.float32)
        nc.vector.memset(ones20_col[:], 1.0)

        ttT = consts.tile([128, NT], dt.float32)
        nc.sync.dma_start(out=ttT[:NT, :], in_=transT.ap())
        expT_sb = consts.tile([128, NT], dt.float32)
        nc.scalar.activation(out=expT_sb[:NT, :], in_=ttT[:NT, :], func=AF.Exp)
        tcolB = consts.tile([128, 1], dt.float32)
        nc.sync.dma_start(out=tcolB[:NT, :], in_=trans_colB.ap())
        outb_sb = consts.tile([128, 1], dt.float32)
        nc.sync.dma_start(out=outb_sb[:NT, :], in_=out_b_col.ap())
        wgT_sb = consts.tile([128, 4 * NT], dt.bfloat16)
        nc.sync.dma_start(out=wgT_sb[:], in_=wgT.ap())
        owT_sb = consts.tile([128, 4 * NT], dt.bfloat16)
        nc.sync.dma_start(out=owT_sb[:], in_=out_wT.ap())
        lnbk_sb = consts.tile([128, 4], dt.bfloat16)
        nc.sync.dma_start(out=lnbk_sb[:], in_=ln_b_k.ap())
        lastoh_sb = consts.tile([128, BC], dt.bfloat16)
        nc.sync.dma_start(out=lastoh_sb[:NT, :], in_=last_oh.ap())
        tfl_sb = consts.tile([128, 4], dt.bfloat16)
        nc.sync.dma_start(out=tfl_sb[:], in_=trans_flat.ap())
        eps_sb = consts.tile([128, 1], dt.float32)
        nc.vector.memset(eps_sb[:], 1e-5)

        beff_sb = consts.tile([128, 1], dt.float32)
        s_sb = consts.tile([128, 1], dt.float32)
        with tc.tile_pool(name="init_ps", bufs=2, space="PSUM") as init_ps:
            be_ps = init_ps.tile([128, 1], dt.float32)
            for k in range(4):
                nc.tensor.matmul(be_ps[:NT, :], owT_sb[:, k * NT:(k + 1) * NT],
                                 lnbk_sb[:, k:k + 1], start=(k == 0), stop=(k == 3))
            nc.vector.tensor_add(out=beff_sb[:NT, :], in0=be_ps[:NT, :],
                                 in1=outb_sb[:NT, :])
            s_ps = init_ps.tile([128, 1], dt.float32)
            for k in range(4):
                nc.tensor.matmul(s_ps[:NT, :], wgT_sb[:, k * NT:(k + 1) * NT],
                                 ones_col_bf[:], start=(k == 0), stop=(k == 3))
            nc.vector.tensor_copy(out=s_sb[:NT, :], in_=s_ps[:NT, :])

        # weights
        wih_sb, whh_sb, bias_sb = {}, {}, {}
        for c in CHAINS:
            wih_sb[c] = wpool.tile([128, KT_IN[c] * 8 * 128], dt.bfloat16, tag=f"wih{c}", name=f"wih{c}")
            nc.sync.dma_start(out=wih_sb[c][:], in_=wih[c].ap())
            whh_sb[c] = wpool.tile([128, 2 * 8 * 128], dt.bfloat16, tag=f"whh{c}", name=f"whh{c}")
            nc.sync.dma_start(out=whh_sb[c][:], in_=whh[c].ap())
            bias_sb[c] = wpool.tile([128, 8], dt.float32, tag=f"bias{c}", name=f"bias{c}")
            nc.sync.dma_start(out=bias_sb[c][:], in_=bias[c].ap())

        # ---------- phase 1: embedding gather + transpose -> x0T ----------
        with tc.tile_pool(name="gat", bufs=4) as gat, \
             tc.tile_pool(name="gat_ps", bufs=4, space="PSUM") as gat_ps:
            for kk in range(ntok // 128):
                idx = gat.tile([128, 1], dt.int32, tag="idx")
                nc.sync.dma_start(out=idx[:],
                                  in_=sent_tm.ap()[kk * 128:(kk + 1) * 128, :])
                xt = gat.tile([128, D], dt.float32, tag="xt")
                nc.gpsimd.indirect_dma_start(
                    out=xt[:], out_offset=None, in_=embed.ap(),
                    in_offset=bass.IndirectOffsetOnAxis(ap=idx[:, :1], axis=0))
                for d in range(2):
                    tp = gat_ps.tile([128, 128], dt.float32, tag="tp")
                    nc.tensor.transpose(tp[:], xt[:, d * 128:(d + 1) * 128], ident[:])
                    xo = gat.tile([128, 128], dt.bfloat16, tag="xo")
                    nc.vector.tensor_copy(out=xo[:], in_=tp[:])
                    nc.sync.dma_start(out=x0T[d].ap()[:, kk * 128:(kk + 1) * 128],
                                      in_=xo[:])

        if stage >= 2:
            with tc.tile_pool(name="xstr", bufs=3) as xstr, \
                 tc.tile_pool(name="xp_ps", bufs=2, space="PSUM") as xp_ps, \
                 tc.tile_pool(name="xp_f", bufs=2) as xp_f, \
                 tc.tile_pool(name="xp_b", bufs=2) as xp_b, \
                 tc.tile_pool(name="gps_f", bufs=2, space="PSUM") as gps_f, \
                 tc.tile_pool(name="gps_b", bufs=2, space="PSUM") as gps_b, \
                 tc.tile_pool(name="sp_f", bufs=3) as sp_f, \
                 tc.tile_pool(name="sp_b", bufs=3) as sp_b, \
                 tc.tile_pool(name="hc_f", bufs=3) as hc_f, \
                 tc.tile_pool(name="hc_b", bufs=3) as hc_b, \
                 tc.tile_pool(name="hst_f", bufs=2) as hst_f, \
                 tc.tile_pool(name="hst_b", bufs=2) as hst_b, \
                 tc.tile_pool(name="mpool", bufs=2) as mpool:

                pools = {
                    "f": dict(xp=xp_f, gps=gps_f, sp=sp_f, hc=hc_f, hst=hst_f),
                    "b": dict(xp=xp_b, gps=gps_b, sp=sp_b, hc=hc_b, hst=hst_b),
                }

                def xp_segment(c, layer_in, seg):
                    dd = c[2]
                    ktn = KT_IN[c]
                    xts = []
                    for k in range(ktn):
                        xt = xstr.tile([128, 512], dt.bfloat16, tag=f"xs{dd}{k}")
                        nc.sync.dma_start(
                            out=xt[:],
                            in_=layer_in[k].ap()[:, seg * 512:(seg + 1) * 512])
                        xts.append(xt)
                    xp_sb = pools[dd]["xp"].tile([128, SEG * 128], dt.bfloat16,
                                                 tag=f"xp{dd}")
                    xp_v = xp_sb[:].rearrange("p (tl m b) -> p tl m b", m=8, b=16)
                    for m in range(8):
                        ps = xp_ps.tile([128, 512], dt.float32, tag="xpps")
                        for k in range(ktn):
                            nc.tensor.matmul(
                                ps[:],
                                wih_sb[c][:, (k * 8 + m) * 128:(k * 8 + m + 1) * 128],
                                xts[k][:], start=(k == 0), stop=(k == ktn - 1))
                        nc.vector.tensor_scalar(
                            out=xp_v[:, :, m, :],
                            in0=ps[:].rearrange("p (tl b) -> p tl b", b=16),
                            scalar1=bias_sb[c][:, m:m + 1], scalar2=None,
                            op0=ALU.add)
                    return xp_sb

                def emit_step(st, tl):
                    c = st["c"]
                    dd = c[2]
                    P = pools[dd]
                    rev = st["rev"]
                    gp = P["gps"].tile([128, 128], dt.float32, tag=f"g{dd}")
                    hT = st["hT"]
                    for m in range(8):
                        for k in range(2):
                            nc.tensor.matmul(
                                gp[:, m * 16:(m + 1) * 16],
                                whh_sb[c][:, (k * 8 + m) * 128:(k * 8 + m + 1) * 128],
                                hT[:, k * 16:(k + 1) * 16],
                                start=(k == 0), stop=(k == 1))
                    gsb = P["sp"].tile([128, 128], dt.float32, tag=f"gsb{dd}")
                    nc.vector.tensor_add(out=gsb[:], in0=gp[:],
                                         in1=st["xp"][:, tl * 128:(tl + 1) * 128])
                    acts = P["sp"].tile([128, 128], dt.float32, tag=f"acts{dd}")
                    nc.scalar.activation(out=acts[:, 0:96], in_=gsb[:, 0:96],
                                         func=AF.Sigmoid)
                    nc.scalar.activation(out=acts[:, 96:128], in_=gsb[:, 96:128],
                                         func=AF.Tanh)
                    t1 = P["sp"].tile([128, 32], dt.float32, tag=f"t1{dd}")
                    nc.vector.tensor_mul(out=t1[:], in0=acts[:, 0:32],
                                         in1=acts[:, 96:128])
                    c2 = P["sp"].tile([128, 32], dt.float32, tag=f"c2{dd}")
                    nc.vector.tensor_mul(out=c2[:], in0=acts[:, 32:64], in1=st["cT"][:])
                    cT = P["hc"].tile([128, 32], dt.float32, tag=f"c{dd}")
                    nc.vector.tensor_add(out=cT[:], in0=c2[:], in1=t1[:])
                    if rev:
                        cm = P["hc"].tile([128, 32], dt.float32, tag=f"cm{dd}")
                        nc.vector.tensor_mul(out=cm[:], in0=cT[:],
                                             in1=st["msk"][:, tl * 32:(tl + 1) * 32])
                        cT = cm
                    st["cT"] = cT
                    tch = P["sp"].tile([128, 32], dt.float32, tag=f"tc{dd}")
                    nc.scalar.activation(out=tch[:], in_=cT[:], func=AF.Tanh)
                    hT_new = st["hs"][:, tl * 32:(tl + 1) * 32]
                    nc.vector.tensor_mul(out=hT_new, in0=acts[:, 64:96], in1=tch[:])
                    st["hT"] = hT_new

                def run_layer(layer, layer_in):
                    cf, cb = f"l{layer}f", f"l{layer}b"
                    sts = {}
                    for c, rev in ((cf, False), (cb, True)):
                        dd = c[2]
                        h0 = pools[dd]["hc"].tile([128, 32], dt.bfloat16, tag=f"h0{dd}")
                        nc.vector.memset(h0[:], 0.0)
                        c0 = pools[dd]["hc"].tile([128, 32], dt.float32, tag=f"c{dd}")
                        nc.vector.memset(c0[:], 0.0)
                        sts[c] = dict(c=c, rev=rev, hT=h0[:], cT=c0, xp=None,
                                      hs=None, msk=None)
                    sts[cf]["xp"] = xp_segment(cf, layer_in, 0)
                    sts[cb]["xp"] = xp_segment(cb, layer_in, nseg - 1)
                    for si in range(nseg):
                        seg_f, seg_b = si, nseg - 1 - si
                        sts[cf]["hs"] = pools["f"]["hst"].tile(
                            [128, SEG * 32], dt.bfloat16, tag="hsf", name="hsf")
                        sts[cb]["hs"] = pools["b"]["hst"].tile(
                            [128, SEG * 32], dt.bfloat16, tag="hsb", name="hsb")
                        msk = mpool.tile([128, SEG * 32], dt.bfloat16, tag="msk")
                        nc.sync.dma_start(
                            out=msk[:],
                            in_=mask_scan.ap()[:, seg_b * SEG * 32:(seg_b + 1) * SEG * 32])
                        sts[cb]["msk"] = msk
                        for tli in range(SEG):
                            emit_step(sts[cf], tli)
                            emit_step(sts[cb], SEG - 1 - tli)
                        for c, seg in ((cf, seg_f), (cb, seg_b)):
                            hv = sts[c]["hs"][:].rearrange(
                                "p (tl k b) -> p tl k b", k=2, b=16)
                            for k in range(2):
                                nc.sync.dma_start(
                                    out=hbuf[c][k].ap()[:, seg * 512:(seg + 1) * 512]
                                    .rearrange("p (tl b) -> p tl b", b=16),
                                    in_=hv[:, :, k, :])
                        if si + 1 < nseg:
                            sts[cf]["xp"] = xp_segment(cf, layer_in, si + 1)
                            sts[cb]["xp"] = xp_segment(cb, layer_in, nseg - 2 - si)

                run_layer(0, x0T)
                if stage >= 3:
                    l1in = [hbuf["l0f"][0], hbuf["l0f"][1],
                            hbuf["l0b"][0], hbuf["l0b"][1]]
                    run_layer(1, l1in)

        # ---------- phase 4: LN + feats + gold accumulation ----------
        if stage >= 3:
            feats_sb = fpool.tile([128, ntok], dt.float32)   # rows 0:20
            gold_sb = fpool.tile([128, ntok], dt.float32)    # row 0
            h2k = [hbuf["l1f"][0], hbuf["l1f"][1], hbuf["l1b"][0], hbuf["l1b"][1]]
            with tc.tile_pool(name="fwork", bufs=3) as fwork, \
                 tc.tile_pool(name="fps", bufs=1, space="PSUM") as fps:
                for chk in range(ntok // 512):
                    sl = slice(chk * 512, (chk + 1) * 512)
                    hts = []
                    for k in range(4):
                        ht = fwork.tile([128, 512], dt.bfloat16, tag=f"h2s{k}")
                        nc.sync.dma_start(out=ht[:], in_=h2k[k].ap()[:, sl])
                        hts.append(ht)
                    fw_ps = fps.tile([128, 512], dt.float32, tag="fw")
                    for k in range(4):
                        nc.tensor.matmul(fw_ps[:NT, :], wgT_sb[:, k * NT:(k + 1) * NT],
                                         hts[k][:], start=(k == 0), stop=(k == 3))
                    mu_ps = fps.tile([128, 512], dt.float32, tag="mu")
                    for k in range(4):
                        nc.tensor.matmul(mu_ps[:1, :], ones_col_bf[:], hts[k][:],
                                         start=(k == 0), stop=(k == 3))
                    sq_ps = fps.tile([128, 512], dt.float32, tag="sq")
                    for k in range(4):
                        sq = fwork.tile([128, 512], dt.bfloat16, tag="sqs")
                        nc.vector.tensor_mul(out=sq[:], in0=hts[k][:], in1=hts[k][:])
                        nc.tensor.matmul(sq_ps[:1, :], ones_col_bf[:], sq[:],
                                         start=(k == 0), stop=(k == 3))
                    mu = fwork.tile([128, 512], dt.float32, tag="mus")
                    nc.vector.tensor_scalar_mul(mu[:1, :], mu_ps[:1, :], 1.0 / HD)
                    var = fwork.tile([128, 512], dt.float32, tag="vars")
                    nc.vector.tensor_scalar_mul(var[:1, :], sq_ps[:1, :], 1.0 / HD)
                    mu2 = fwork.tile([128, 512], dt.float32, tag="mu2s")
                    nc.vector.tensor_mul(out=mu2[:1, :], in0=mu[:1, :], in1=mu[:1, :])
                    nc.vector.tensor_tensor(out=var[:1, :], in0=var[:1, :],
                                            in1=mu2[:1, :], op=ALU.subtract)
                    nc.vector.tensor_scalar(out=var[:1, :], in0=var[:1, :],
                                            scalar1=0.0, scalar2=None, op0=ALU.max)
                    std = fwork.tile([128, 512], dt.float32, tag="stds")
                    nc.scalar.activation(out=std[:1, :], in_=var[:1, :], func=AF.Sqrt,
                                         bias=eps_sb[:1, :])
                    rstd = fwork.tile([128, 512], dt.float32, tag="rstds")
                    nc.vector.reciprocal(out=rstd[:1, :], in_=std[:1, :])
                    mr = fwork.tile([128, 512], dt.float32, tag="mrs")
                    nc.vector.tensor_mul(out=mr[:1, :], in0=mu[:1, :], in1=rstd[:1, :])
                    R_ps = fps.tile([128, 512], dt.float32, tag="R")
                    nc.tensor.matmul(R_ps[:NT, :], ones_row[:1, :NT], rstd[:1, :],
                                     start=True, stop=True)
                    MR_ps = fps.tile([128, 512], dt.float32, tag="MR")
                    nc.tensor.matmul(MR_ps[:NT, :], ones_row[:1, :NT], mr[:1, :],
                                     start=True, stop=True)
                    Rsb = fwork.tile([128, 512], dt.float32, tag="Rsb")
                    nc.vector.tensor_copy(out=Rsb[:NT, :], in_=R_ps[:NT, :])
                    tmp = fwork.tile([128, 512], dt.float32, tag="ft1")
                    nc.vector.tensor_mul(out=tmp[:NT, :], in0=fw_ps[:NT, :],
                                         in1=Rsb[:NT, :])
                    tmp2 = fwork.tile([128, 512], dt.float32, tag="ft2")
                    nc.vector.tensor_scalar(out=tmp2[:NT, :], in0=MR_ps[:NT, :],
                                            scalar1=s_sb[:NT, :], scalar2=None,
                                            op0=ALU.mult)
                    nc.vector.tensor_tensor(out=tmp[:NT, :], in0=tmp[:NT, :],
                                            in1=tmp2[:NT, :], op=ALU.subtract)
                    nc.vector.tensor_scalar(out=feats_sb[:NT, sl], in0=tmp[:NT, :],
                                            scalar1=beff_sb[:NT, :], scalar2=None,
                                            op0=ALU.add)
                    # gold: emissions + transition pairs
                    ohtile = fwork.tile([128, 512], dt.bfloat16, tag="ohts")
                    nc.sync.dma_start(out=ohtile[:NT, :], in_=oht.ap()[:, sl])
                    gm = fwork.tile([128, 512], dt.float32, tag="gms")
                    nc.vector.tensor_mul(out=gm[:NT, :], in0=feats_sb[:NT, sl],
                                         in1=ohtile[:NT, :])
                    g_ps = fps.tile([128, 512], dt.float32, tag="gld")
                    nc.tensor.matmul(g_ps[:1, :], ones20_col[:NT, :], gm[:NT, :],
                                     start=True, stop=False)
                    for k in range(4):
                        o2 = fwork.tile([128, 512], dt.bfloat16, tag="oh2s")
                        nc.sync.dma_start(out=o2[:],
                                          in_=oh2.ap()[k * 128:(k + 1) * 128, sl])
                        nc.tensor.matmul(g_ps[:1, :], tfl_sb[:, k:k + 1], o2[:],
                                         start=False, stop=(k == 3))
                    nc.vector.tensor_copy(out=gold_sb[:1, sl], in_=g_ps[:1, :])
                if dbg and dbg_feats is not None:
                    for chk in range(ntok // 512):
                        sl = slice(chk * 512, (chk + 1) * 512)
                        nc.sync.dma_start(out=dbg_feats.ap()[:, sl],
                                          in_=feats_sb[:NT, sl])

        # ---------- phase 5: CRF + outputs ----------
        if stage >= 4:
            with tc.tile_pool(name="crf", bufs=3) as crf, \
                 tc.tile_pool(name="crfm", bufs=2) as crfm, \
                 tc.tile_pool(name="crf_ps", bufs=2, space="PSUM") as crf_ps:
                alpha = fpool.tile([128, BC], dt.float32)
                nc.vector.tensor_scalar(out=alpha[:NT, :], in0=feats_sb[:NT, 0:BC],
                                        scalar1=tcolB[:NT, :], scalar2=None,
                                        op0=ALU.add)
                cmask_sb = None
                for t in range(1, t_steps):
                    off = (t * BC) % 512
                    if cmask_sb is None or off == 0:
                        base = (t * BC) // 512 * 512
                        cmask_sb = crfm.tile([128, 512], dt.uint8, tag="cmsk")
                        nc.sync.dma_start(out=cmask_sb[:NT, :],
                                          in_=crf_mask.ap()[:, base:base + 512])
                    bc_ps = crf_ps.tile([128, BC], dt.float32, tag="bc")
                    nc.tensor.matmul(bc_ps[:NT, :], ones_row[:1, :NT], alpha[0:1, :],
                                     start=True, stop=True)
                    sub = crf.tile([128, BC], dt.float32, tag="sub")
                    nc.vector.tensor_tensor(out=sub[:NT, :], in0=alpha[:NT, :],
                                            in1=bc_ps[:NT, :], op=ALU.subtract)
                    E = crf.tile([128, BC], dt.float32, tag="E")
                    nc.scalar.activation(out=E[:NT, :], in_=sub[:NT, :], func=AF.Exp)
                    S_ps = crf_ps.tile([128, BC], dt.float32, tag="S")
                    nc.tensor.matmul(S_ps[:NT, :], expT_sb[:NT, :NT], E[:NT, :],
                                     start=True, stop=True)
                    Sc = crf.tile([128, BC], dt.float32, tag="Sc")
                    nc.vector.tensor_scalar(out=Sc[:NT, :], in0=S_ps[:NT, :],
                                            scalar1=1e-30, scalar2=None, op0=ALU.max)
                    lg = crf.tile([128, BC], dt.float32, tag="lg")
                    nc.scalar.activation(out=lg[:NT, :], in_=Sc[:NT, :], func=AF.Ln)
                    a1 = crf.tile([128, BC], dt.float32, tag="a1")
                    nc.vector.tensor_add(out=a1[:NT, :], in0=lg[:NT, :],
                                         in1=feats_sb[:NT, t * BC:(t + 1) * BC])
                    a2 = crf.tile([128, BC], dt.float32, tag="a2")
                    nc.vector.tensor_add(out=a2[:NT, :], in0=a1[:NT, :],
                                         in1=bc_ps[:NT, :])
                    nc.vector.copy_predicated(out=alpha[:NT, :],
                                              mask=cmask_sb[:NT, off:off + BC],
                                              data=a2[:NT, :])
                # fscore
                bc2 = crf_ps.tile([128, BC], dt.float32, tag="bc")
                nc.tensor.matmul(bc2[:NT, :], ones_row[:1, :NT], alpha[0:1, :],
                                 start=True, stop=True)
                sub2 = crf.tile([128, BC], dt.float32, tag="sub")
                nc.vector.tensor_tensor(out=sub2[:NT, :], in0=alpha[:NT, :],
                                        in1=bc2[:NT, :], op=ALU.subtract)
                nc.vector.tensor_scalar(out=sub2[:NT, :], in0=sub2[:NT, :],
                                        scalar1=ttT[:NT, END:END + 1], scalar2=None,
                                        op0=ALU.add)
                Ex = crf.tile([128, BC], dt.float32, tag="E")
                nc.scalar.activation(out=Ex[:NT, :], in_=sub2[:NT, :], func=AF.Exp)
                se_ps = crf_ps.tile([128, BC], dt.float32, tag="S")
                nc.tensor.matmul(se_ps[:1, :], ones20_col[:NT, :], Ex[:NT, :],
                                 start=True, stop=True)
                lg2 = crf.tile([128, BC], dt.float32, tag="lg")
                nc.scalar.activation(out=lg2[:1, :], in_=se_ps[:1, :], func=AF.Ln)
                fsc = crf.tile([128, BC], dt.float32, tag="a1")
                nc.vector.tensor_add(out=fsc[:1, :], in0=lg2[:1, :], in1=alpha[0:1, :])
                nc.sync.dma_start(out=fscore_out.ap(), in_=fsc[:1, :])
                # gold total
                lt = crf.tile([128, BC], dt.float32, tag="a2")
                nc.vector.tensor_scalar(out=lt[:NT, :], in0=lastoh_sb[:NT, :],
                                        scalar1=ttT[:NT, END:END + 1], scalar2=None,
                                        op0=ALU.mult)
                le_ps = crf_ps.tile([128, BC], dt.float32, tag="bc")
                nc.tensor.matmul(le_ps[:1, :], ones20_col[:NT, :], lt[:NT, :],
                                 start=True, stop=True)
                gred = crf.tile([128, BC], dt.float32, tag="sub")
                nc.vector.tensor_reduce(
                    out=gred[:1, :],
                    in_=gold_sb[:1, :].rearrange("p (t b) -> p b t", b=BC),
                    axis=mybir.AxisListType.X, op=ALU.add)
                gtot = crf.tile([128, BC], dt.float32, tag="E")
                nc.vector.tensor_add(out=gtot[:1, :], in0=gred[:1, :],
                                     in1=le_ps[:1, :])
                nc.sync.dma_start(out=gold_out.ap(), in_=gtot[:1, :])

    _split_waits(nc)
    return nc


# ====================== host side ======================

def prep_core_inputs(core, inputs, t_steps=T):
    b0 = core * BC
    sl = slice(b0, b0 + BC)
    sent = np.asarray(inputs["sent"])[sl].astype(np.int32)
    seq_len = np.asarray(inputs["seq_len"])[sl].astype(np.int64)
    tags = np.asarray(inputs["tags"])[sl].astype(np.int64)
    ntok = t_steps * BC

    m = {}
    m["sent_tm"] = np.ascontiguousarray(
        sent[:, :t_steps].T).reshape(ntok, 1)
    m["embed"] = np.ascontiguousarray(np.asarray(inputs["embed"], dtype=np.float32))

    for c in CHAINS:
        lay, d = c[1], c[2]
        w_ih = np.asarray(inputs[f"w_ih_l{lay}{d}"], dtype=np.float32)[_GATE_PERM]
        w_hh = np.asarray(inputs[f"w_hh_l{lay}{d}"], dtype=np.float32)[_GATE_PERM]
        b_c = (np.asarray(inputs[f"b_ih_l{lay}{d}"], dtype=np.float32)
               + np.asarray(inputs[f"b_hh_l{lay}{d}"], dtype=np.float32))[_GATE_PERM]
        m[f"wih_{c}"] = _pack_lhsT(w_ih).astype(_BF16)
        m[f"whh_{c}"] = _pack_lhsT(w_hh).astype(_BF16)
        m[f"bias_{c}"] = np.ascontiguousarray(b_c.reshape(8, 128).T)

    valid = (np.arange(t_steps)[:, None] < seq_len[None, :])     # [T, 16]
    msk = np.repeat(valid[:, None, :], 2, axis=1).reshape(t_steps * 32)
    m["mask_scan"] = np.broadcast_to(
        msk.astype(_BF16)[None, :], (128, t_steps * 32)).copy()

    trans = np.asarray(inputs["trans"], dtype=np.float32)
    m["transT"] = np.ascontiguousarray(trans.T)
    m["trans_colB"] = np.ascontiguousarray(trans[:, BEGIN:BEGIN + 1])
    ln_g = np.asarray(inputs["ln_g"], dtype=np.float32)
    ln_b = np.asarray(inputs["ln_b"], dtype=np.float32)
    out_w = np.asarray(inputs["out_w"], dtype=np.float32)
    wg = out_w * ln_g[None, :]
    m["wgT"] = np.ascontiguousarray(
        wg.T.reshape(4, 128, NT).transpose(1, 0, 2).reshape(128, 4 * NT)).astype(_BF16)
    m["out_wT"] = np.ascontiguousarray(
        out_w.T.reshape(4, 128, NT).transpose(1, 0, 2).reshape(128, 4 * NT)).astype(_BF16)
    m["ln_b_k"] = np.ascontiguousarray(ln_b.reshape(4, 128).T).astype(_BF16)
    m["out_b_col"] = np.asarray(inputs["out_b"], dtype=np.float32).reshape(NT, 1)

    tt = np.arange(t_steps)
    cols = (tt[:, None] * BC + np.arange(BC)[None, :])
    ohtv = np.zeros((NT, ntok), dtype=np.float32)
    ohtv[tags.T[:t_steps], cols] = valid.astype(np.float32)
    m["oht"] = ohtv.astype(_BF16)

    tg_prev = np.concatenate([np.full((BC, 1), BEGIN, np.int64), tags[:, :-1]], 1)
    pair = (tags * NT + tg_prev).T[:t_steps]
    oh2v = np.zeros((512, ntok), dtype=np.float32)
    oh2v[pair, cols] = valid.astype(np.float32)
    m["oh2"] = oh2v.astype(_BF16)
    tf = np.zeros(512, np.float32)
    tf[:400] = trans.reshape(-1)
    m["trans_flat"] = np.ascontiguousarray(tf.reshape(4, 128).T).astype(_BF16)

    m["crf_mask"] = np.broadcast_to(
        valid.reshape(-1).astype(np.uint8)[None, :], (NT, ntok)).copy()
    last_tag = tags[np.arange(BC), seq_len - 1]
    loh = np.zeros((NT, BC), np.float32)
    loh[last_tag, np.arange(BC)] = 1.0
    m["last_oh"] = loh.astype(_BF16)
    return m


_PROG_CACHE = {}


def _get_program():
    if "full" not in _PROG_CACHE:
        _PROG_CACHE["full"] = build_program(T, stage=4, dbg=False)
    return _PROG_CACHE["full"]


def kernel(**inputs):
    from concourse.bass_utils import run_bass_kernel_spmd

    nc = _get_program()
    in_maps = [prep_core_inputs(c, inputs) for c in range(NCORES)]
    res = run_bass_kernel_spmd(nc, in_maps, list(range(NCORES)))
    tot = 0.0
    for c in range(NCORES):
        fs = res.results[c]["fscore_out"].reshape(-1)
        gd = res.results[c]["gold_out"].reshape(-1)
        tot += float(np.sum(fs - gd))
    return np.float32(tot / B)
